# revision 6
# baseline (speedup 1.0000x reference)
"""APPNP (MLP + 10x personalized-pagerank propagation + log_softmax) on 8 TRN2
NeuronCores.

Sharding: nodes split across 8 cores (12500 each, padded to 12544 = 98 blocks
of 128 lanes). The propagation table zs = dinv*z is replicated via AllGather
each iteration, stored bf16 with FOUR node rows packed per 512B table row
("quad") so the full table is addressable by int16 dma_gather indices.

Per iteration each core gathers its in-edges' source quads as a dense message
stream (sorted by dest block / source quarter / dest lane, padded to shared
per-block-quarter group sizes so all 8 cores run an identical instruction
stream). The segment sum runs on the TensorEngine: per 128-message tile a
static one-hot "Sel" matrix (precomputed on host, streamed bf16 from HBM)
maps messages to destination lanes, accumulating in PSUM; the DVE folds the
four quarter column groups and applies the z/zs update.
"""
import os
import sys

import numpy as np


def _ensure_paths():
    try:
        import concourse  # noqa: F401
        return
    except ImportError:
        pass
    for p in ("/opt/trn_rl_repo", "/root/.axon_site/_ro/trn_rl_repo"):
        if os.path.isdir(p) and p not in sys.path:
            sys.path.insert(0, p)


N_NODES = 100000
IN_C, HID_C, OUT_C = 512, 256, 64
NCORES = 8
SH = 12500
P = 128
NB = 98
SHP = NB * P          # 12544
K_ITERS = 10
ALPHA = 0.1
QUADS = NCORES * SHP // 4   # 25088
ZQ = 25080            # all-dummy quad -> always zeros
TPI = 4               # message tiles (128 msgs) per dma_gather instruction
NBUF = 8              # gather staging ring (instructions in flight)
SELB = 16             # Sel matrices per DMA batch

LAST_EXEC_NS = None
LAST_RESULT = None


def _host_prep(x, edge_index):
    import ml_dtypes
    row = np.asarray(edge_index[0], dtype=np.int64)
    col = np.asarray(edge_index[1], dtype=np.int64)
    deg = np.bincount(row, minlength=N_NODES).astype(np.int64)
    dinv = np.where(deg > 0, 1.0 / np.sqrt(np.maximum(deg, 1)), 0.0).astype(np.float32)

    orders = []
    pos_of = np.zeros(N_NODES, dtype=np.int64)
    for c in range(NCORES):
        nodes = np.arange(c * SH, (c + 1) * SH)
        order = nodes[np.argsort(-deg[nodes], kind="stable")]
        orders.append(order)
        pos_of[order] = np.arange(SH)
    g_of = pos_of + (np.arange(N_NODES) // SH) * SHP

    # per-core edges keyed (block, quarter, lane)
    core_of_edge = row // SH
    edat = []
    cnts = np.zeros((NCORES, NB, 4), dtype=np.int64)
    for c in range(NCORES):
        m = core_of_edge == c
        l = pos_of[row[m]]                      # local dest pos
        sg = g_of[col[m]]                       # source padded-global id
        q = (sg % 4).astype(np.int64)
        j = l // P
        key = (j * 4 + q) * P + (l % P)         # sort by (block, quarter, lane)
        o = np.argsort(key, kind="stable")
        l, sg, q, j = l[o], sg[o], q[o], j[o]
        edat.append((l, sg, q, j))
        np.add.at(cnts[c], (j, q), 1)

    G = cnts.max(axis=0)                        # [NB, 4] shared group sizes
    Mblk = G.sum(axis=1)
    TM = ((Mblk + P - 1) // P).astype(np.int64)  # tiles per block
    PadM = TM * P
    gq0 = np.zeros((NB, 4), dtype=np.int64)
    gq0[:, 1:] = np.cumsum(G, axis=1)[:, :-1]
    MO = np.zeros(NB + 1, dtype=np.int64)
    np.cumsum(PadM, out=MO[1:])
    TOTM = int(MO[-1])

    # gather instruction list (shared): (block, tile0, ntiles)
    instrs = []
    tile_instr = {}
    for j in range(NB):
        t = 0
        while t < TM[j]:
            nt = int(min(TPI, TM[j] - t))
            for tt in range(t, t + nt):
                tile_instr[(j, tt)] = len(instrs)
            instrs.append((j, t, nt))
            t += nt
    NI = len(instrs)

    # matmul descriptors (shared): (block, tile, q, r0, r1, start, stop, instr)
    descs = []
    for j in range(NB):
        firstq = {qq: True for qq in range(4)}
        lastd_q = {}
        for t in range(int(TM[j])):
            lo, hi = t * P, t * P + P
            for qq in range(4):
                a = max(lo, int(gq0[j, qq]))
                b = min(hi, int(gq0[j, qq] + G[j, qq]))
                if a < b:
                    descs.append([j, t, qq, a - lo, b - lo, firstq[qq], False,
                                  tile_instr[(j, t)]])
                    lastd_q[qq] = len(descs) - 1
                    firstq[qq] = False
        for qq, di in lastd_q.items():
            descs[di][6] = True
    NSEL = len(descs)
    NSB = (NSEL + SELB - 1) // SELB

    last_desc_of_instr = {}
    for di, d in enumerate(descs):
        last_desc_of_instr[d[7]] = di

    qs_of_block = [[qq for qq in range(4) if G[j, qq] > 0] for j in range(NB)]

    # per-core gather indices and Sel matrices
    gidx_all, sel_all = [], []
    for c in range(NCORES):
        l, sg, q, j = edat[c]
        keys = j * 4 + q
        _, grp_starts_idx = np.unique(keys, return_index=True)
        start_of = np.zeros(len(l), dtype=np.int64)
        start_of[grp_starts_idx] = grp_starts_idx
        np.maximum.accumulate(start_of, out=start_of)
        within = np.arange(len(l)) - start_of
        mpos = MO[j] + gq0[j, q] + within
        idx_flat = np.full(TOTM, ZQ, dtype=np.int16)
        idx_flat[mpos] = (sg // 4).astype(np.int16)
        wraps = []
        for (jb, t0, nt) in instrs:
            n = P * nt
            seg = idx_flat[MO[jb] + t0 * P: MO[jb] + t0 * P + n]
            w = np.zeros((16, n // 16), dtype=np.int16)
            w[np.arange(n) % 16, np.arange(n) // 16] = seg
            wraps.append(w)
        gidx_all.append(np.ascontiguousarray(
            np.tile(np.concatenate(wraps, axis=1), (8, 1))))
        dest_lane = np.full(TOTM, -1, dtype=np.int64)
        dest_lane[mpos] = l % P
        sel = np.zeros((NSEL, P, P), dtype=np.float32)
        for di, (jb, t, qq, r0, r1, st, sp, ii) in enumerate(descs):
            base = MO[jb] + t * P
            rows = np.arange(r0, r1)
            lanes = dest_lane[base + rows]
            valid = lanes >= 0
            sel[di, rows[valid], lanes[valid]] = 1.0
        sel_all.append(np.ascontiguousarray(sel.astype(ml_dtypes.bfloat16)))

    xT_all, dinvL_all, A_all, C_all, A2_all = [], [], [], [], []
    xf = np.asarray(x, dtype=np.float32)
    for c in range(NCORES):
        xp = np.zeros((SHP, IN_C), dtype=np.float32)
        xp[:SH] = xf[orders[c]]
        xT_all.append(np.ascontiguousarray(xp.T))
        dl = np.zeros(SHP, dtype=np.float32)
        dl[:SH] = dinv[orders[c]]
        dg = np.ascontiguousarray(dl.reshape(NB, P).T)
        dinvL_all.append(dg)
        A_all.append(np.ascontiguousarray((1 - ALPHA) * dg * dg))
        C_all.append(np.ascontiguousarray(ALPHA * dg))
        A2_all.append(np.ascontiguousarray((1 - ALPHA) * dg))

    return dict(orders=orders, instrs=instrs, descs=descs, NSB=NSB,
                last_desc_of_instr=last_desc_of_instr, qs_of_block=qs_of_block,
                gidx=gidx_all, sel=sel_all, xT=xT_all, dinvL=dinvL_all,
                A=A_all, C=C_all, A2=A2_all)


def _build_graph(prep):
    import concourse.bacc as bacc
    from concourse import bass, mybir
    from concourse.library_config import mlp as mlp_lib
    from contextlib import ExitStack

    instrs, descs = prep["instrs"], prep["descs"]
    NSB = prep["NSB"]
    last_desc_of_instr = prep["last_desc_of_instr"]
    qs_of_block = prep["qs_of_block"]
    NI = len(instrs)
    NSEL = len(descs)
    GW = 8 * sum(nt for (_, _, nt) in instrs)
    WLOADS = 11 * 16

    nc = bacc.Bacc("TRN2", num_swdge_queues=4)
    f32, bf16, i16 = mybir.dt.float32, mybir.dt.bfloat16, mybir.dt.int16
    Relu = mybir.ActivationFunctionType.Relu
    Exp = mybir.ActivationFunctionType.Exp
    Ln = mybir.ActivationFunctionType.Ln
    add = mybir.AluOpType.add

    xT_d = nc.dram_tensor("xT", [IN_C, SHP], f32, kind="ExternalInput")
    W1_d = nc.dram_tensor("W1", [IN_C, HID_C], f32, kind="ExternalInput")
    W2_d = nc.dram_tensor("W2", [HID_C, HID_C], f32, kind="ExternalInput")
    W3_d = nc.dram_tensor("W3", [HID_C, OUT_C], f32, kind="ExternalInput")
    b1_d = nc.dram_tensor("b1w", [P, 2], f32, kind="ExternalInput")
    b2_d = nc.dram_tensor("b2w", [P, 2], f32, kind="ExternalInput")
    b3_d = nc.dram_tensor("b3r", [P, OUT_C], f32, kind="ExternalInput")
    gidx_d = nc.dram_tensor("gidx", [P, GW], i16, kind="ExternalInput")
    sel_d = nc.dram_tensor("selm", [NSEL, P, P], bf16, kind="ExternalInput")
    dinv_d = nc.dram_tensor("dinvL", [P, NB], f32, kind="ExternalInput")
    A_d = nc.dram_tensor("Ac", [P, NB], f32, kind="ExternalInput")
    C_d = nc.dram_tensor("Cc", [P, NB], f32, kind="ExternalInput")
    A2_d = nc.dram_tensor("A2c", [P, NB], f32, kind="ExternalInput")
    out_d = nc.dram_tensor("out", [SHP, OUT_C], f32, kind="ExternalOutput")

    zsh = nc.dram_tensor("zsh", [SHP // 4, 256], bf16)
    zstab = nc.dram_tensor("zstab", [QUADS, 256], bf16, addr_space="Shared")

    def dview(t, dims, off=0):
        return bass.AP(t[:].tensor, off, dims)

    with ExitStack() as _ctx:
        e = _ctx.enter_context
        gidx_s = e(nc.sbuf_tensor("gidx_s", [P, GW], i16))
        h_s = e(nc.sbuf_tensor("h_s", [P, NB, OUT_C], f32))
        zs_s = e(nc.sbuf_tensor("zs_s", [P, NB, OUT_C], bf16))
        out_s = e(nc.sbuf_tensor("out_s", [P, NB, OUT_C], f32))
        stag_s = e(nc.sbuf_tensor("stag_s", [P, NBUF, TPI, 256], bf16))
        selb_s = e(nc.sbuf_tensor("selb_s", [P, 2, SELB, P], bf16))
        t0_s = e(nc.sbuf_tensor("t0_s", [P, OUT_C], f32))
        t1_s = e(nc.sbuf_tensor("t1_s", [P, OUT_C], f32))
        t2_s = e(nc.sbuf_tensor("t2_s", [P, OUT_C], f32))
        xsb = e(nc.sbuf_tensor("xsb", [P, 2, 4, P], f32))
        W1s = e(nc.sbuf_tensor("W1s", [P, 4, 2, P], f32))
        W2s = e(nc.sbuf_tensor("W2s", [P, 2, 2, P], f32))
        W3s = e(nc.sbuf_tensor("W3s", [P, 2, OUT_C], f32))
        b1s = e(nc.sbuf_tensor("b1s", [P, 2], f32))
        b2s = e(nc.sbuf_tensor("b2s", [P, 2], f32))
        b3s = e(nc.sbuf_tensor("b3s", [P, OUT_C], f32))
        h1s = e(nc.sbuf_tensor("h1s", [P, 2, P], f32))
        h2s = e(nc.sbuf_tensor("h2s", [P, 2, P], f32))
        dinv_s = e(nc.sbuf_tensor("dinv_s", [P, NB], f32))
        A_s = e(nc.sbuf_tensor("A_s", [P, NB], f32))
        C_s = e(nc.sbuf_tensor("C_s", [P, NB], f32))
        A2_s = e(nc.sbuf_tensor("A2_s", [P, NB], f32))
        mx_s = e(nc.sbuf_tensor("mx_s", [P, 1], f32))
        sum_s = e(nc.sbuf_tensor("sum_s", [P, 1], f32))
        ls_s = e(nc.sbuf_tensor("ls_s", [P, 1], f32))
        e_s = e(nc.sbuf_tensor("e_s", [P, OUT_C], f32))
        ps1 = [e(nc.psum_tensor("ps1a", [P, P], f32)),
               e(nc.psum_tensor("ps1b", [P, P], f32))]
        ps2 = [e(nc.psum_tensor("ps2a", [P, P], f32)),
               e(nc.psum_tensor("ps2b", [P, P], f32))]
        ps3 = [e(nc.psum_tensor("ps3a", [P, OUT_C], f32)),
               e(nc.psum_tensor("ps3b", [P, OUT_C], f32))]
        psg = [e(nc.psum_tensor("psga", [P, 256], f32)),
               e(nc.psum_tensor("psgb", [P, 256], f32))]
        x_sem = e(nc.semaphore("x_sem"))
        w_sem = e(nc.semaphore("w_sem"))
        g_sem = e(nc.semaphore("g_sem"))
        v_sem = e(nc.semaphore("v_sem"))
        u_sem = e(nc.semaphore("u_sem"))
        d_sem = e(nc.semaphore("d_sem"))
        cc_sem = e(nc.semaphore("cc_sem"))
        sb_sem = e(nc.semaphore("sb_sem"))
        fold_sem = e(nc.semaphore("fold_sem"))
        mm1_sem = e(nc.semaphore("mm1_sem"))
        act1_sem = e(nc.semaphore("act1_sem"))
        mm2_sem = e(nc.semaphore("mm2_sem"))
        act2_sem = e(nc.semaphore("act2_sem"))
        mm3_sem = e(nc.semaphore("mm3_sem"))
        hz_sem = e(nc.semaphore("hz_sem"))
        sm1_sem = e(nc.semaphore("sm1_sem"))
        se_sem = e(nc.semaphore("se_sem"))
        sm_sem = e(nc.semaphore("sm_sem"))
        block = e(nc.Block())

        # ------------- sync engine: inputs + Sel stream + final store ------
        @block.sync
        def _(sync):
            sync.dma_start(W1s[:], dview(W1_d, [[HID_C, P], [P * HID_C, 4],
                                                [P, 2], [1, P]])).then_inc(w_sem, 16)
            sync.dma_start(W2s[:], dview(W2_d, [[HID_C, P], [P * HID_C, 2],
                                                [P, 2], [1, P]])).then_inc(w_sem, 16)
            sync.dma_start(W3s[:], dview(W3_d, [[OUT_C, P], [P * OUT_C, 2],
                                                [1, OUT_C]])).then_inc(w_sem, 16)
            sync.dma_start(b1s[:], b1_d[:]).then_inc(w_sem, 16)
            sync.dma_start(b2s[:], b2_d[:]).then_inc(w_sem, 16)
            sync.dma_start(b3s[:], b3_d[:]).then_inc(w_sem, 16)
            sync.dma_start(dinv_s[:], dinv_d[:]).then_inc(w_sem, 16)
            sync.dma_start(A_s[:], A_d[:]).then_inc(w_sem, 16)
            sync.dma_start(C_s[:], C_d[:]).then_inc(w_sem, 16)
            sync.dma_start(A2_s[:], A2_d[:]).then_inc(w_sem, 16)
            sync.dma_start(gidx_s[:], gidx_d[:]).then_inc(w_sem, 16)
            for b in range(NB):
                if b >= 2:
                    sync.wait_ge(mm1_sem, 2 * (b - 2) + 2)
                sync.dma_start(
                    xsb[:, b % 2],
                    dview(xT_d, [[SHP, P], [P * SHP, 4], [1, P]], off=b * P),
                ).then_inc(x_sem, 16)
            for k in range(K_ITERS):
                for b in range(NSB):
                    gb = k * NSB + b
                    if gb >= 2:
                        pb = gb - 2
                        di = min((pb % NSB) * SELB + SELB - 1, NSEL - 1)
                        ii = descs[di][7]
                        sync.wait_ge(v_sem, (pb // NSB) * NI + ii + 1)
                    nsel = min(SELB, NSEL - b * SELB)
                    sync.dma_start(
                        dview(selb_s, [[2 * SELB * P, P], [P, nsel], [1, P]],
                              off=(gb % 2) * SELB * P),
                        dview(sel_d, [[P, P], [P * P, nsel], [1, P]],
                              off=b * SELB * P * P),
                    ).then_inc(sb_sem, 16)
            sync.wait_ge(sm_sem, NB)
            sync.dma_start(
                dview(out_d, [[OUT_C, P], [P * OUT_C, NB], [1, OUT_C]]),
                out_s[:],
            ).then_inc(x_sem, 16)
            sync.wait_ge(x_sem, 16 * (NB + 1))

        # ------------- tensor engine: MLP + segment-sum matmuls ------------
        @block.tensor
        def _(tensor):
            tensor.wait_ge(w_sem, WLOADS)
            for b in range(NB):
                tensor.wait_ge(x_sem, 16 * (b + 1))
                for mh in range(2):
                    for kk in range(4):
                        mm = tensor.matmul(ps1[mh][:], W1s[:, kk, mh],
                                           xsb[:, b % 2, kk],
                                           start=(kk == 0), stop=(kk == 3))
                    mm.then_inc(mm1_sem, 1)
                tensor.wait_ge(act1_sem, 2 * b + 2)
                for mh in range(2):
                    for kk in range(2):
                        mm = tensor.matmul(ps2[mh][:], W2s[:, kk, mh],
                                           h1s[:, kk],
                                           start=(kk == 0), stop=(kk == 1))
                    mm.then_inc(mm2_sem, 1)
                tensor.wait_ge(act2_sem, 2 * b + 2)
                if b >= 2:
                    tensor.wait_ge(hz_sem, b - 1)
                for kk in range(2):
                    mm = tensor.matmul(ps3[b % 2][:], h2s[:, kk], W3s[:, kk],
                                       start=(kk == 0), stop=(kk == 1))
                mm.then_inc(mm3_sem, 1)
            for k in range(K_ITERS):
                curj = -1
                previ = -1
                for di, (j, t, qq, r0, r1, st, sp, ii) in enumerate(descs):
                    gi = k * NI + ii
                    gb = k * NSB + di // SELB
                    if j != curj:
                        curj = j
                        if k * NB + j >= 2:
                            tensor.wait_ge(fold_sem, k * NB + j - 1)
                    if di % SELB == 0:
                        tensor.wait_ge(sb_sem, 16 * (gb + 1))
                    if ii != previ:
                        tensor.wait_ge(g_sem, 16 * (gi + 1))
                        previ = ii
                    mm = tensor.matmul(
                        psg[j % 2][:, 64 * qq:64 * qq + 64],
                        selb_s[:, gb % 2, di % SELB],
                        stag_s[:, gi % NBUF, t - instrs[ii][1],
                               64 * qq:64 * qq + 64],
                        start=st, stop=sp, skip_group_check=True)
                    if last_desc_of_instr.get(ii) == di:
                        mm.then_inc(v_sem, 1)
                previ = -1

        # ------------- scalar engine: relus + softmax ----------------------
        @block.scalar
        def _(scalar):
            scalar.wait_ge(w_sem, WLOADS)
            for b in range(NB):
                for mh in range(2):
                    scalar.wait_ge(mm1_sem, 2 * b + mh + 1)
                    scalar.activation(h1s[:, mh], ps1[mh][:], Relu,
                                      bias=b1s[:, mh:mh + 1]).then_inc(act1_sem, 1)
                for mh in range(2):
                    scalar.wait_ge(mm2_sem, 2 * b + mh + 1)
                    scalar.activation(h2s[:, mh], ps2[mh][:], Relu,
                                      bias=b2s[:, mh:mh + 1]).then_inc(act2_sem, 1)
            for b in range(NB):
                scalar.wait_ge(sm1_sem, b + 1)
                scalar.activation(e_s[:], t1_s[:], Exp, accum_out=sum_s[:, :1])
                scalar.activation(ls_s[:], sum_s[:], Ln).then_inc(se_sem, 1)

        # ------------- gpsimd: gathers, collectives, zs dumps --------------
        @block.gpsimd
        def _(gpsimd):
            gpsimd.load_library(mlp_lib)
            gpsimd.wait_ge(w_sem, WLOADS)
            d = 0
            for k in range(K_ITERS):
                gpsimd.wait_ge(u_sem, (k + 1) * NB)
                gpsimd.dma_start(
                    dview(zsh, [[OUT_C, P], [P * OUT_C, NB], [1, OUT_C]]),
                    zs_s[:],
                ).then_inc(d_sem, 16)
                d += 16
                gpsimd.wait_ge(d_sem, d)
                if k > 0:
                    gpsimd.wait_ge(g_sem, 16 * NI * k)
                gpsimd.collective_compute(
                    "AllGather", mybir.AluOpType.bypass,
                    replica_groups=[list(range(NCORES))],
                    ins=[zsh[:]], outs=[zstab[:]],
                ).then_inc(cc_sem, 1)
                gpsimd.wait_ge(cc_sem, k + 1)
                gw = 0
                for i, (j, t0g, nt) in enumerate(instrs):
                    gi = k * NI + i
                    if gi >= NBUF:
                        gpsimd.wait_ge(v_sem, gi - NBUF + 1)
                    gpsimd.dma_gather(
                        stag_s[:, gi % NBUF, :nt], zstab[:],
                        gidx_s[:, gw:gw + 8 * nt],
                        P * nt, P * nt, 256,
                        queue_num=gi % 4,
                    ).then_inc(g_sem, 16)
                    gw += 8 * nt

        # ------------- vector engine: h/zs init, folds, updates, softmax ---
        @block.vector
        def _(vector):
            import concourse.mybir as mb
            vector.wait_ge(w_sem, WLOADS)
            for b in range(NB):
                vector.wait_ge(mm3_sem, b + 1)
                vector.tensor_tensor(out=h_s[:, b], in0=ps3[b % 2][:],
                                     in1=b3s[:], op=add).then_inc(hz_sem, 1)
                vector.tensor_scalar_mul(out=zs_s[:, b], in0=h_s[:, b],
                                         scalar1=dinv_s[:, b:b + 1]).then_inc(u_sem, 1)
            lastinstr_of_block = {}
            for i, (j, t0g, nt) in enumerate(instrs):
                lastinstr_of_block[j] = i
            for k in range(K_ITERS):
                last = k == K_ITERS - 1
                for j in range(NB):
                    qs = qs_of_block[j]
                    if qs:
                        vector.wait_ge(v_sem, k * NI + lastinstr_of_block[j] + 1)
                        pg = psg[j % 2]
                        if len(qs) == 1:
                            u_ap = pg[:, 64 * qs[0]:64 * qs[0] + 64]
                        else:
                            vector.tensor_scalar_mul(
                                out=t0_s[:], in0=pg[:, 64 * qs[0]:64 * qs[0] + 64],
                                scalar1=1.0)
                            for qq in qs[1:]:
                                vector.tensor_tensor(
                                    out=t0_s[:], in0=t0_s[:],
                                    in1=pg[:, 64 * qq:64 * qq + 64], op=add)
                            u_ap = t0_s[:]
                        vector.tensor_scalar_mul(
                            out=t1_s[:], in0=u_ap,
                            scalar1=(A2_s if last else A_s)[:, j:j + 1]
                        ).then_inc(fold_sem, 1)
                        if last:
                            vector.tensor_scalar_mul(out=t2_s[:], in0=h_s[:, j],
                                                     scalar1=ALPHA)
                            vector.tensor_tensor(out=h_s[:, j], in0=t1_s[:],
                                                 in1=t2_s[:], op=add).then_inc(u_sem, 1)
                        else:
                            vector.tensor_scalar_mul(out=t2_s[:], in0=h_s[:, j],
                                                     scalar1=C_s[:, j:j + 1])
                            vector.tensor_tensor(out=zs_s[:, j], in0=t1_s[:],
                                                 in1=t2_s[:], op=add).then_inc(u_sem, 1)
                    else:
                        if last:
                            vector.tensor_scalar_mul(
                                out=h_s[:, j], in0=h_s[:, j],
                                scalar1=ALPHA).then_inc(u_sem, 1)
                        else:
                            vector.tensor_scalar_mul(
                                out=zs_s[:, j], in0=h_s[:, j],
                                scalar1=C_s[:, j:j + 1]).then_inc(u_sem, 1)
                        vector.tensor_scalar_mul(
                            out=t1_s[:], in0=t1_s[:],
                            scalar1=1.0).then_inc(fold_sem, 1)
            for b in range(NB):
                vector.tensor_reduce(out=mx_s[:], in_=h_s[:, b],
                                     axis=mb.AxisListType.X, op=mb.AluOpType.max)
                vector.tensor_scalar_sub(out=t1_s[:], in0=h_s[:, b],
                                         scalar1=mx_s[:, :1]).then_inc(sm1_sem, 1)
                vector.wait_ge(se_sem, b + 1)
                vector.tensor_scalar_sub(out=out_s[:, b], in0=t1_s[:],
                                         scalar1=ls_s[:, :1]).then_inc(sm_sem, 1)

    nc.compile()
    return nc


def kernel(x, edge_index, W1, b1, W2, b2, W3, b3):
    global LAST_EXEC_NS, LAST_RESULT
    _ensure_paths()
    from concourse.bass_utils import run_bass_kernel_spmd

    x = np.asarray(x, dtype=np.float32)
    prep = _host_prep(x, np.asarray(edge_index))
    nc = _build_graph(prep)

    W1 = np.asarray(W1, np.float32)
    W2 = np.asarray(W2, np.float32)
    W3 = np.asarray(W3, np.float32)
    b1 = np.asarray(b1, np.float32)
    b2 = np.asarray(b2, np.float32)
    b3 = np.asarray(b3, np.float32)
    b1w = np.ascontiguousarray(b1.reshape(2, P).T)
    b2w = np.ascontiguousarray(b2.reshape(2, P).T)
    b3r = np.ascontiguousarray(np.tile(b3[None, :], (P, 1)))

    in_maps = []
    for c in range(NCORES):
        in_maps.append({
            "xT": prep["xT"][c],
            "W1": W1, "W2": W2, "W3": W3,
            "b1w": b1w, "b2w": b2w, "b3r": b3r,
            "gidx": prep["gidx"][c],
            "selm": prep["sel"][c],
            "dinvL": prep["dinvL"][c],
            "Ac": prep["A"][c], "Cc": prep["C"][c], "A2c": prep["A2"][c],
        })

    trace = os.environ.get("APPNP_TRACE", "0") == "1"
    if trace:
        try:
            import profile_shim
            profile_shim.install()
        except Exception:
            trace = False
    res = run_bass_kernel_spmd(nc, in_maps, list(range(NCORES)), trace=trace)
    LAST_EXEC_NS = res.exec_time_ns
    LAST_RESULT = res

    out = np.zeros((N_NODES, OUT_C), dtype=np.float32)
    for c in range(NCORES):
        out[prep["orders"][c]] = res.results[c]["out"][:SH]
    return out
